# revision 1
# baseline (speedup 1.0000x reference)
"""FISTA sparse-coding encoder kernel for Trainium2 (8 NeuronCores).

Problem: x [2,10,20480] f32, Drr/Dtheta [40] f32.
  D = normalized dictionary [10, 161]
  A = I - D^T D / L,  DtY = D^T Y / L,  lam = gamma / L
  40 FISTA iterations: xn = softshrink(A @ y + DtY); y = xn + m (xn - x_old)
  output sparsecode [2, 161, 20480].

Design ("u-form"):
  - Data-parallel over columns: Y reshaped to [10, 40960]; 5120 columns/core.
  - Augmented matmul: u_i = Aaug contraction over [x_i(161 rows); Y(10 rows)]
    = A x_i + DtY, so DtY needs no separate pass.  Aaug = vstack([A.T, D/L])
    [171,161], chunked 128+43 (contraction) x 128+33 (out rows).
  - Momentum identity: A y_i + DtY = (1+m)(A x_i + DtY) - m (A x_{i-1} + DtY)
    = (1+m) u_i - m u_{i-1}.  So per iteration per column group:
      PE:  8 matmuls  -> u_i in PSUM
      DVE: one fused custom op   xn = shrink(u*s0 + u_old*s1, lam)
      ACT: copy u PSUM -> SBUF (becomes u_old of the next iteration)
    Single u_old buffer: the fused op reads u_old before ACT overwrites it.
  - Matmul inputs in float32r (full-rate fp32 path on the PE); rounded on
    write by the producing DVE op.
"""

import numpy as np

# ---------------------------------------------------------------- constants
B, T, N_POLES, P = 2, 10, 40, 20480
MAX_ITER = 40
GAMMA = 0.01
K = 4 * N_POLES + 1          # 161
NCORES = 8
NCOLS = B * P // NCORES      # 5120 columns per core
BLK = 512                    # matmul free dim (one PSUM bank)
GRP = 1024                   # fused-op group (2 banks)
NGRP = NCOLS // GRP          # 5
PS_BUFS = 2                  # PSUM pool buffers
KH = 128                     # head rows
KT = K - KH                  # 33 tail rows
KA = KT + T                  # 43 = tail rows + glued Y rows

_cache = {}


# ------------------------------------------------------------- custom DVE op
def _register_shrinkmom2():
    """out = relu(w - C2) - relu(-w - C2)  with  w = in0*s0 + in1.

    s0 = (1+m); in1 must already hold -m * u_{i-1} (the ScalarE evacuation
    copy applies that scale); imm2 = lam.  Exactly 8 ALU stages.
    """
    import concourse.dve_ops as dve_ops
    from concourse.dve_spec import Spec, Src0, Src1, C0, C2, Zero, relu, lower
    from concourse.dve_spec import _has_src1 as has_src1
    from concourse.dve_uop import DveOpSpec

    name = "ANT_SHRINKMOM2_FISTA"
    if any(op.name == name for op in dve_ops.OPS):
        return next(op for op in dve_ops.OPS if op.name == name)

    w = Src0 * C0 + Src1
    spec = Spec(
        body=relu(w - C2) - relu(Zero - w - C2),
        reference=lambda in0, in1, s0=1.0, s1=0.0, imm2=0.0: (
            lambda ww: (np.maximum(ww - imm2, 0.0)
                        - np.maximum(-ww - imm2, 0.0)).astype(np.float32)
        )(in0 * s0 + in1),
    )
    op = dve_ops.DveOp(name, spec, subdim=False, uops_sha={})
    dve_ops.OPS.append(op)
    dve_ops.CUSTOM_DVE_SPECS[name] = spec
    dve_ops._SUB_OPCODE_FOR_NAME[name] = (
        dve_ops._CUSTOM_DVE_ROW_BASE + len(dve_ops.OPS) - 1
    )
    for ver in ("v3", "v4"):
        compiled = DveOpSpec(
            name=name,
            opcode=dve_ops.get_dve_sub_opcode(name),
            uops=lower(spec, ver=ver),
            rd1_en=has_src1(spec),
        )
        op.uops_sha[ver] = compiled.sha(ver)
    return op


# ------------------------------------------------------------ host constants
def _host_constants(Drr, Dtheta):
    r = Drr.astype(np.float64)
    th = Dtheta.astype(np.float64)
    i = np.arange(T, dtype=np.float64)[:, None]
    pr = r[None, :] ** i
    sgn = np.where(np.arange(T)[:, None] % 2 == 0, 1.0, -1.0)
    c = np.cos(i * th[None, :])
    s = np.sin(i * th[None, :])
    ones = np.ones((T, 1))
    dic = np.concatenate([ones, pr * c, sgn * pr * c, pr * s, sgn * pr * s], axis=1)
    G = np.linalg.norm(dic, axis=0)
    G = np.where(G == 0, np.sqrt(float(T)), G)
    D = (dic / G).astype(np.float32)            # [T, K]

    D64 = D.astype(np.float64)
    DtD = D64.T @ D64
    L = float(np.linalg.norm(DtD))              # Frobenius
    A = np.eye(K) - DtD / L                     # [K, K]
    lam = float(GAMMA / L)

    Aaug = np.concatenate([A.T, D64 / L], axis=0).astype(np.float32)  # [171, K]

    # momentum coefficients m_i = (t_i - 1)/t_{i+1}, t_0 = 1
    ms = []
    t = 1.0
    for _ in range(MAX_ITER):
        t_new = (1.0 + np.sqrt(1.0 + 4.0 * t * t)) / 2.0
        ms.append((t - 1.0) / t_new)
        t = t_new
    return Aaug, lam, ms


# ------------------------------------------------------------- bass program
def _build_program():
    import concourse.mybir as mybir
    import concourse.tile as tile
    from concourse import bacc

    fused_op = _register_shrinkmom2()

    f32 = mybir.dt.float32
    f32r = mybir.dt.float32r

    nc = bacc.Bacc("TRN2", target_bir_lowering=False, debug=False,
                   num_devices=NCORES)

    ycols = nc.dram_tensor("ycols", [T, NCOLS], f32, kind="ExternalInput")
    d_l1a = nc.dram_tensor("l1a", [KH, KH], f32, kind="ExternalInput")
    d_l1b = nc.dram_tensor("l1b", [KH, KT], f32, kind="ExternalInput")
    d_l2a = nc.dram_tensor("l2a", [KA, KH], f32, kind="ExternalInput")
    d_l2b = nc.dram_tensor("l2b", [KA, KT], f32, kind="ExternalInput")
    d_l0a = nc.dram_tensor("l0a", [KA, KH], f32, kind="ExternalInput")
    d_l0b = nc.dram_tensor("l0b", [KA, KT], f32, kind="ExternalInput")
    out = nc.dram_tensor("out", [K, NCOLS], f32, kind="ExternalOutput")

    lam, ms = _cache["consts_meta"]

    with tile.TileContext(nc) as tc:
        with (
            tc.tile_pool(name="state", bufs=1) as st,
            tc.tile_pool(name="wts", bufs=1) as wts,
            tc.tile_pool(name="psH", bufs=PS_BUFS, space="PSUM") as psH,
            tc.tile_pool(name="psT", bufs=PS_BUFS, space="PSUM") as psT,
        ):
            # ---- persistent state -------------------------------------
            xH = [st.tile([KH, NCOLS], f32r, tag=f"xH{b}", name=f"xH{b}")
                  for b in range(2)]
            xT = [st.tile([KA, NCOLS], f32r, tag=f"xT{b}", name=f"xT{b}")
                  for b in range(2)]          # rows KT: KA hold glued Y
            uoH = st.tile([KH, NCOLS], f32, tag="uoH", name="uoH")
            uoT = st.tile([KT, NCOLS], f32, tag="uoT", name="uoT")

            # fp32 staging for DMA'd weights -> rounded f32r copies
            lt1a = wts.tile([KH, KH], f32, tag="lt1a", name="lt1a")
            lt1b = wts.tile([KH, KT], f32, tag="lt1b", name="lt1b")
            lt2a = wts.tile([KA, KH], f32, tag="lt2a", name="lt2a")
            lt2b = wts.tile([KA, KT], f32, tag="lt2b", name="lt2b")
            l1a = wts.tile([KH, KH], f32r, tag="l1a", name="l1a")
            l1b = wts.tile([KH, KT], f32r, tag="l1b", name="l1b")
            l2a = wts.tile([KA, KH], f32r, tag="l2a", name="l2a")
            l2b = wts.tile([KA, KT], f32r, tag="l2b", name="l2b")
            lt0a = wts.tile([KA, KH], f32, tag="lt0a", name="lt0a")
            lt0b = wts.tile([KA, KT], f32, tag="lt0b", name="lt0b")
            l0a = wts.tile([KA, KH], f32r, tag="l0a", name="l0a")
            l0b = wts.tile([KA, KT], f32r, tag="l0b", name="l0b")
            # K=10 f32r matmuls crash the PE; pad the DtY contraction to 43
            # rows (Y on top, zeros below) — 43 is hardware-validated.
            ygl = wts.tile([KA, NCOLS], f32r, tag="ygl", name="ygl")

            nc.sync.dma_start(lt1a[:], d_l1a[:])
            nc.sync.dma_start(lt1b[:], d_l1b[:])
            nc.sync.dma_start(lt2a[:], d_l2a[:])
            nc.sync.dma_start(lt2b[:], d_l2b[:])
            nc.sync.dma_start(lt0a[:], d_l0a[:])
            nc.sync.dma_start(lt0b[:], d_l0b[:])
            nc.scalar.copy(l0a[:], lt0a[:])
            nc.scalar.copy(l0b[:], lt0b[:])
            nc.scalar.copy(l1a[:], lt1a[:])
            nc.scalar.copy(l1b[:], lt1b[:])
            nc.scalar.copy(l2a[:], lt2a[:])
            nc.scalar.copy(l2b[:], lt2b[:])

            # ---- init.  Iteration 0 computes u_0 = DtY from ygl alone, so
            # x buffers never need zero-filling: buf1 is fully written at
            # it=0, buf0 at it=1; only the Y glue rows must be populated.
            nc.gpsimd.memset(uoH[:], 0.0)
            nc.gpsimd.memset(uoT[:], 0.0)
            with tc.tile_pool(name="init", bufs=1) as ip:
                gstage = ip.tile([KA, NCOLS], f32, tag="gstage", name="gstage")
                nc.gpsimd.memset(gstage[0:32, :], 0.0)
                nc.gpsimd.memset(gstage[32:KA, :], 0.0)
                nc.sync.dma_start(gstage[0:T, :], ycols[:, :])
                nc.sync.dma_start(gstage[KT:KA, :], ycols[:, :])
                nc.vector.tensor_copy(ygl[0:32, :], gstage[0:32, :])
                nc.vector.tensor_copy(ygl[32:KA, :], gstage[32:KA, :])
                nc.scalar.copy(xT[0][32:KA, :], gstage[32:KA, :])
                nc.gpsimd.tensor_copy(xT[1][32:KA, :], gstage[32:KA, :])

            def mm(ps, lhsT, rhs, start, stop):
                nc.tensor.matmul(ps, lhsT, rhs, start=start, stop=stop)

            for it in range(MAX_ITER):
                m_prev = ms[it - 1] if it > 0 else 0.0
                s0 = float(1.0 + m_prev)
                xc_h = xH[it % 2]            # x_it
                xc_t = xT[it % 2]
                xn_h = xH[(it + 1) % 2]      # will hold x_{it+1}
                xn_t = xT[(it + 1) % 2]

                for g in range(NGRP):
                    gs = slice(g * GRP, (g + 1) * GRP)

                    wh = psH.tile([KH, GRP], mybir.dt.float32, tag="wh",
                                  name="wh")
                    wt = psT.tile([KT, GRP], mybir.dt.float32, tag="wt",
                                  name="wt")

                    for b in range(GRP // BLK):
                        bs = slice(g * GRP + b * BLK, g * GRP + (b + 1) * BLK)
                        ps = slice(b * BLK, (b + 1) * BLK)
                        if it == 0:
                            # u_0 = A*0 + DtY: only the D/L rows contribute
                            mm(wh[:, ps], l0a[:], ygl[:, bs], True, True)
                            mm(wt[:, ps], l0b[:], ygl[:, bs], True, True)
                        else:
                            mm(wh[:, ps], l1a[:], xc_h[:, bs], True, False)
                            mm(wh[:, ps], l2a[:], xc_t[:, bs], False, True)
                            mm(wt[:, ps], l1b[:], xc_h[:, bs], True, False)
                            mm(wt[:, ps], l2b[:], xc_t[:, bs], False, True)

                    # fused momentum + soft-threshold (PSUM+SBUF -> f32r
                    # SBUF); in1 already holds -m_prev * u_{i-1}
                    nc.vector._custom_dve(fused_op, out=xn_h[:, gs],
                                          in0=wh[:], in1=uoH[:, gs],
                                          s0=s0, imm2=float(lam))
                    nc.vector._custom_dve(fused_op, out=xn_t[0:KT, gs],
                                          in0=wt[:], in1=uoT[:, gs],
                                          s0=s0, imm2=float(lam))

                    if it == MAX_ITER - 1:
                        nc.sync.dma_start(out[0:KH, gs],
                                          xn_h[:, gs].bitcast(f32))
                        nc.sync.dma_start(out[KH:K, gs],
                                          xn_t[0:KT, gs].bitcast(f32))
                    else:
                        # u_i (scaled by -m_it) becomes next iteration's
                        # u_old term, written after the fused op above has
                        # consumed the previous contents
                        nc.scalar.mul(uoH[:, gs], wh[:], float(-ms[it]))
                        nc.scalar.mul(uoT[:, gs], wt[:], float(-ms[it]))
    nc.finalize()
    return nc


def _get_program(lam, ms):
    key = (round(lam, 12), tuple(round(m, 9) for m in ms))
    if _cache.get("key") != key:
        _cache["consts_meta"] = (lam, ms)
        _cache["nc"] = _build_program()
        _cache["key"] = key
    return _cache["nc"]


# ------------------------------------------------------------------- kernel
def kernel(x, Drr, Dtheta):
    from concourse.bass_utils import run_bass_kernel_spmd

    Aaug, lam, ms = _host_constants(Drr, Dtheta)
    nc = _get_program(lam, ms)

    l1a = np.ascontiguousarray(Aaug[0:KH, 0:KH])
    l1b = np.ascontiguousarray(Aaug[0:KH, KH:K])
    l2a = np.ascontiguousarray(Aaug[KH:KH + KA, 0:KH])
    l2b = np.ascontiguousarray(Aaug[KH:KH + KA, KH:K])
    l0a = np.zeros((KA, KH), np.float32)
    l0b = np.zeros((KA, KT), np.float32)
    l0a[0:T] = Aaug[K:K + T, 0:KH]
    l0b[0:T] = Aaug[K:K + T, KH:K]

    xc = np.ascontiguousarray(
        np.transpose(x.astype(np.float32), (1, 0, 2)).reshape(T, B * P))

    in_maps = []
    for c in range(NCORES):
        in_maps.append({
            "ycols": np.ascontiguousarray(xc[:, c * NCOLS:(c + 1) * NCOLS]),
            "l1a": l1a, "l1b": l1b, "l2a": l2a, "l2b": l2b,
            "l0a": l0a, "l0b": l0b,
        })

    res = run_bass_kernel_spmd(nc, in_maps, core_ids=list(range(NCORES)))
    _cache["last_res"] = res
    full = np.concatenate([r["out"] for r in res.results], axis=1)  # [K, B*P]
    return np.ascontiguousarray(
        full.reshape(K, B, P).transpose(1, 0, 2)).astype(np.float32)


if __name__ == "__main__":
    x = np.random.randn(B, T, P).astype(np.float32)
    Drr = np.random.rand(N_POLES).astype(np.float32)
    Dtheta = np.random.rand(N_POLES).astype(np.float32)
    o = kernel(x, Drr, Dtheta)
    print(o.shape, o.dtype)



# revision 15
# speedup vs baseline: 1.2514x; 1.2514x over previous
"""FISTA sparse-coding encoder kernel for Trainium2 (8 NeuronCores).

Problem: x [2,10,20480] f32, Drr/Dtheta [40] f32.
  D = normalized dictionary [10, 161]
  A = I - D^T D / L,  DtY = D^T Y / L,  lam = gamma / L
  40 FISTA iterations: xn = softshrink(A @ y + DtY); y = xn + m (xn - x_old)
  output sparsecode [2, 161, 20480].

Design ("u-form", v4: fp16 matmuls + PSUM-packed tails):
  - Data-parallel over columns: Y reshaped to [10, 40960]; 5120 columns/core.
  - Momentum identity: A y_i + DtY = (1+m) u_i - m u_{i-1} with
    u_i = A x_i + DtY, so each iteration needs one matmul pass over x_i and
    one fused elementwise op  xn = shrink(s0*u_i - m*u_{i-1}, lam).
  - Output rows split 118 (head) + 43 (tail).  Contraction split 128 + 43:
    block 1 = [x rows 0..117 ; Y glue rows] (DtY via the glue), block 2 =
    x rows 118..160 packed at partition bases 0/64 of xT.
  - Tail packing: the two 512-column halves of a 1024-column group write
    their 43 tail rows into ONE [107, 512] PSUM tile at partition offsets 0
    and 64 (PE quad-tile col positions; half A's tail out-block is 64 wide
    with 21 zero weight columns so the pad partitions are initialized).
    The tail DVE/ACT ops then process 1024 columns in 512 free-cycles,
    cutting elementwise busy/iter from 2N to 1.5N.  PSUM partition offsets
    require 16-bit matmul operands (walrus rejects f32r quad-tiling), so
    x-state and weights are fp16: u stays f32 in PSUM, the momentum
    combines f32 u's, and only x is rounded; measured end-to-end error
    ~9e-3 vs the f32 reference (tolerance 2e-2).
  - Per iteration per group:
      PE:  8 matmuls -> u head [118,1024] + u tail packed [107,512] (f32)
      DVE: fused shrink+momentum from PSUM + raw-u SBUF -> fp16 x state
      ACT: raw copies u -> uoH/uoT (single buffer; DVE reads the old value
           first, write-after-read ordered by the tile framework)
"""

import numpy as np

# ---------------------------------------------------------------- constants
B, T, N_POLES, P = 2, 10, 40, 20480
MAX_ITER = 40
GAMMA = 0.01
K = 4 * N_POLES + 1          # 161
NCORES = 8
NCOLS = B * P // NCORES      # 5120 columns per core
GRP = 1024                   # column group (head psum = 2 banks)
HB = 512                     # half-block (one PSUM bank, matmul free dim)
NGRP = NCOLS // GRP          # 5
KH = 118                     # head out rows (x rows 0..117)
KT = K - KH                  # 43 tail out rows (x rows 118..160)
PCOLS = NCOLS // 2           # 2560 packed tail columns
TB = 64                      # packed-tail half B base partition
XT_P = TB + KT               # 107 partitions of packed tail state

_cache = {}


# ------------------------------------------------------------- custom DVE op
def _register_shrinkmom3():
    """out = relu(w - C2) + min(w + C2, 0)  with  w = in0*s0 + in1*s1.

    Softshrink of an affine combination of two raw tensors:
    s0 = (1+m_prev), s1 = -m_prev, imm2 = lam.  8 ALU stages.
    """
    import concourse.dve_ops as dve_ops
    from concourse.dve_spec import (
        Spec, Src0, Src1, C0, C1, C2, Zero, relu, minn, lower,
    )
    from concourse.dve_spec import _has_src1 as has_src1
    from concourse.dve_uop import DveOpSpec

    name = "ANT_SHRINKMOM3_FISTA"
    if any(op.name == name for op in dve_ops.OPS):
        return next(op for op in dve_ops.OPS if op.name == name)

    w = Src0 * C0 + Src1 * C1
    spec = Spec(
        body=relu(w - C2) + minn(w + C2, Zero),
        reference=lambda in0, in1, s0=1.0, s1=0.0, imm2=0.0: (
            lambda ww: (np.maximum(ww - imm2, 0.0)
                        + np.minimum(ww + imm2, 0.0)).astype(np.float32)
        )(in0 * s0 + in1 * s1),
    )
    op = dve_ops.DveOp(name, spec, subdim=False, uops_sha={})
    dve_ops.OPS.append(op)
    dve_ops.CUSTOM_DVE_SPECS[name] = spec
    dve_ops._SUB_OPCODE_FOR_NAME[name] = (
        dve_ops._CUSTOM_DVE_ROW_BASE + len(dve_ops.OPS) - 1
    )
    for ver in ("v3", "v4"):
        compiled = DveOpSpec(
            name=name,
            opcode=dve_ops.get_dve_sub_opcode(name),
            uops=lower(spec, ver=ver),
            rd1_en=has_src1(spec),
        )
        op.uops_sha[ver] = compiled.sha(ver)
    return op


# ------------------------------------------------------------ host constants
def _host_constants(Drr, Dtheta):
    r = Drr.astype(np.float64)
    th = Dtheta.astype(np.float64)
    i = np.arange(T, dtype=np.float64)[:, None]
    pr = r[None, :] ** i
    sgn = np.where(np.arange(T)[:, None] % 2 == 0, 1.0, -1.0)
    c = np.cos(i * th[None, :])
    s = np.sin(i * th[None, :])
    ones = np.ones((T, 1))
    dic = np.concatenate([ones, pr * c, sgn * pr * c, pr * s, sgn * pr * s], axis=1)
    G = np.linalg.norm(dic, axis=0)
    G = np.where(G == 0, np.sqrt(float(T)), G)
    D = (dic / G).astype(np.float32)            # [T, K]

    D64 = D.astype(np.float64)
    DtD = D64.T @ D64
    L = float(np.linalg.norm(DtD))              # Frobenius
    A = np.eye(K) - DtD / L                     # [K, K] (symmetric)
    lam = float(GAMMA / L)

    # contraction block 1 rows: [A rows 0..117 ; D/L glue rows]  [128, K]
    w1 = np.concatenate([A[0:KH, :], D64 / L], axis=0).astype(np.float32)
    # contraction block 2 rows: A rows 118..160  [43, K]
    w2 = A[KH:K, :].astype(np.float32)

    # momentum coefficients m_i = (t_i - 1)/t_{i+1}, t_0 = 1
    ms = []
    t = 1.0
    for _ in range(MAX_ITER):
        t_new = (1.0 + np.sqrt(1.0 + 4.0 * t * t)) / 2.0
        ms.append((t - 1.0) / t_new)
        t = t_new
    return w1, w2, lam, ms


# ------------------------------------------------------------- bass program
def _build_program():
    import concourse.mybir as mybir
    import concourse.tile as tile
    from concourse import bacc

    fused_op = _register_shrinkmom3()

    f32 = mybir.dt.float32
    f16 = mybir.dt.float16

    nc = bacc.Bacc("TRN2", target_bir_lowering=False, debug=False,
                   num_devices=NCORES)

    ycols = nc.dram_tensor("ycols", [T, NCOLS], f32, kind="ExternalInput")
    d_l1a = nc.dram_tensor("l1a", [128, KH], f32, kind="ExternalInput")
    # tail-out weights for half A carry 21 zero columns so the matmul also
    # zero-fills the pad partitions 43..63 of the packed PSUM tile (free —
    # matmul cost depends only on the moving free size)
    d_l1b = nc.dram_tensor("l1b", [128, TB], f32, kind="ExternalInput")
    d_l2a = nc.dram_tensor("l2a", [KT, KH], f32, kind="ExternalInput")
    d_l2b = nc.dram_tensor("l2b", [KT, TB], f32, kind="ExternalInput")
    out = nc.dram_tensor("out", [K, NCOLS], f32, kind="ExternalOutput")

    lam, ms = _cache["consts_meta"]

    with tile.TileContext(nc) as tc:
        with (
            tc.tile_pool(name="state", bufs=1) as st,
            tc.tile_pool(name="wts", bufs=1) as wts,
            tc.tile_pool(name="psH", bufs=3, space="PSUM") as psH,
            tc.tile_pool(name="psT", bufs=2, space="PSUM") as psT,
        ):
            # ---- persistent state -------------------------------------
            # xH rows 0..117 = x head; rows 118..127 = Y glue (written once)
            xH = [st.tile([128, NCOLS], f16, tag=f"xH{b}", name=f"xH{b}")
                  for b in range(2)]
            # packed tail: half A at partitions 0..42, half B at 64..106
            xT = [st.tile([XT_P, PCOLS], f16, tag=f"xT{b}", name=f"xT{b}")
                  for b in range(2)]
            # raw u_{i-1} staging (single buffer: fused op reads before the
            # ACT copy overwrites)
            uoH = st.tile([KH, NCOLS], f32, tag="uoH", name="uoH")
            uoT = st.tile([XT_P, PCOLS], f32, tag="uoT", name="uoT")
            # f32 copies of the final iterate for the output DMA
            foH = st.tile([KH, NCOLS], f32, tag="foH", name="foH")
            foT = st.tile([XT_P, PCOLS], f32, tag="foT", name="foT")

            # fp32 staging for DMA'd weights -> fp16 copies
            ws1 = wts.tile([128, KH + TB], f32, tag="ws1", name="ws1")
            ws2 = wts.tile([KT, KH + TB], f32, tag="ws2", name="ws2")
            l1a = wts.tile([128, KH], f16, tag="l1a", name="l1a")
            l1b = wts.tile([128, TB], f16, tag="l1b", name="l1b")
            # block-2 weights duplicated at partition bases 0 and 64
            l2a = wts.tile([XT_P, KH], f16, tag="l2a", name="l2a")
            l2b = wts.tile([XT_P, TB], f16, tag="l2b", name="l2b")

            nc.sync.dma_start(ws1[:, 0:KH], d_l1a[:])
            nc.sync.dma_start(ws1[:, KH:KH + TB], d_l1b[:])
            nc.sync.dma_start(ws2[:, 0:KH], d_l2a[:])
            nc.sync.dma_start(ws2[:, KH:KH + TB], d_l2b[:])
            nc.scalar.copy(l1a[:], ws1[:, 0:KH])
            nc.scalar.copy(l1b[:], ws1[:, KH:KH + TB])
            nc.scalar.copy(l2a[0:KT, :], ws2[:, 0:KH])
            nc.scalar.copy(l2b[0:KT, :], ws2[:, KH:KH + TB])
            nc.scalar.copy(l2a[TB:TB + KT, :], ws2[:, 0:KH])
            nc.scalar.copy(l2b[TB:TB + KT, :], ws2[:, KH:KH + TB])

            # ---- init ------------------------------------------------
            # x_0 = 0 (so iteration 0 computes u_0 = DtY from the glue
            # alone); uo = 0.  Zeros + glue-Y staged in f32 and
            # engine-copied (converting) into the fp16 state.
            nc.gpsimd.memset(uoH[:], 0.0)
            nc.gpsimd.memset(uoT[:], 0.0)
            with tc.tile_pool(name="init", bufs=1) as ip:
                zst = ip.tile([128, NCOLS], f32, tag="zst", name="zst")
                nc.gpsimd.memset(zst[:], 0.0)
                nc.sync.dma_start(zst[KH:128, :], ycols[:, :])
                nc.scalar.copy(xH[0][:], zst[:])
                nc.vector.tensor_copy(xH[1][96:128, :], zst[96:128, :])
                nc.gpsimd.tensor_copy(xT[0][:], zst[0:XT_P, 0:PCOLS])

            def mm(ps, lhsT, rhs, start, stop):
                nc.tensor.matmul(ps, lhsT, rhs, start=start, stop=stop,
                                 skip_group_check=True)

            for it in range(MAX_ITER):
                m_prev = ms[it - 1] if it > 0 else 0.0
                s0 = float(1.0 + m_prev)
                s1 = float(-m_prev)
                cur, nxt = it % 2, (it + 1) % 2
                xc_h, xc_t = xH[cur], xT[cur]
                xn_h, xn_t = xH[nxt], xT[nxt]
                last = it == MAX_ITER - 1

                for g in range(NGRP):
                    gs = slice(g * GRP, (g + 1) * GRP)
                    pc = slice(g * HB, (g + 1) * HB)
                    csA = slice(g * GRP, g * GRP + HB)
                    csB = slice(g * GRP + HB, (g + 1) * GRP)

                    wh = psH.tile([KH, GRP], mybir.dt.float32, tag="wh",
                                  name="wh")
                    wt = psT.tile([XT_P, HB], mybir.dt.float32, tag="wt",
                                  name="wt")

                    # half A: contraction block 2 at partition base 0; tail
                    # out-block 64 wide (43 real + 21 zero) -> pads written
                    mm(wh[:, 0:HB], l1a[:], xc_h[:, csA], True, False)
                    mm(wh[:, 0:HB], l2a[0:KT, :], xc_t[0:KT, pc], False, True)
                    mm(wt[0:TB, :], l1b[:], xc_h[:, csA], True, False)
                    mm(wt[0:TB, :], l2b[0:KT, :], xc_t[0:KT, pc], False, True)
                    # half B: tail lands at PSUM partition base 64
                    mm(wh[:, HB:GRP], l1a[:], xc_h[:, csB], True, False)
                    mm(wh[:, HB:GRP], l2a[TB:TB + KT, :],
                       xc_t[TB:TB + KT, pc], False, True)
                    mm(wt[TB:TB + KT, :], l1b[:, 0:KT], xc_h[:, csB],
                       True, False)
                    mm(wt[TB:TB + KT, :], l2b[TB:TB + KT, 0:KT],
                       xc_t[TB:TB + KT, pc], False, True)

                    # fused momentum + soft-threshold (reads uo = u_{i-1}
                    # BEFORE the ACT copies below overwrite it).  The final
                    # iterate goes to f32 tiles for the output DMA instead
                    # of the fp16 state (nothing consumes x_40 on-chip).
                    oh = foH[:, gs] if last else xn_h[0:KH, gs]
                    ot = foT[:, pc] if last else xn_t[:, pc]
                    nc.vector._custom_dve(fused_op, out=oh,
                                          in0=wh[:], in1=uoH[:, gs],
                                          s0=s0, s1=s1, imm2=float(lam))
                    nc.vector._custom_dve(fused_op, out=ot,
                                          in0=wt[:], in1=uoT[:, pc],
                                          s0=s0, s1=s1, imm2=float(lam))

                    if last:
                        nc.sync.dma_start(out[0:KH, gs], foH[:, gs])
                        nc.sync.dma_start(out[KH:K, csA], foT[0:KT, pc])
                        nc.sync.dma_start(out[KH:K, csB],
                                          foT[TB:TB + KT, pc])
                    else:
                        nc.scalar.copy(uoH[:, gs], wh[:])
                        nc.scalar.copy(uoT[:, pc], wt[:])
    nc.finalize()
    return nc


def _get_program(lam, ms):
    key = (round(lam, 12), tuple(round(m, 9) for m in ms))
    if _cache.get("key") != key:
        _cache["consts_meta"] = (lam, ms)
        _cache["nc"] = _build_program()
        _cache["key"] = key
    return _cache["nc"]


# ------------------------------------------------------------------- kernel
def kernel(x, Drr, Dtheta):
    from concourse.bass_utils import run_bass_kernel_spmd

    w1, w2, lam, ms = _host_constants(Drr, Dtheta)
    nc = _get_program(lam, ms)

    l1a = np.ascontiguousarray(w1[:, 0:KH])
    l1b = np.zeros((128, TB), np.float32)
    l1b[:, 0:KT] = w1[:, KH:K]
    l2a = np.ascontiguousarray(w2[:, 0:KH])
    l2b = np.zeros((KT, TB), np.float32)
    l2b[:, 0:KT] = w2[:, KH:K]

    xc = np.ascontiguousarray(
        np.transpose(x.astype(np.float32), (1, 0, 2)).reshape(T, B * P))

    in_maps = []
    for c in range(NCORES):
        in_maps.append({
            "ycols": np.ascontiguousarray(xc[:, c * NCOLS:(c + 1) * NCOLS]),
            "l1a": l1a, "l1b": l1b, "l2a": l2a, "l2b": l2b,
        })

    res = run_bass_kernel_spmd(nc, in_maps, core_ids=list(range(NCORES)))
    _cache["last_res"] = res
    full = np.concatenate([r["out"] for r in res.results], axis=1)  # [K, B*P]
    return np.ascontiguousarray(
        full.reshape(K, B, P).transpose(1, 0, 2)).astype(np.float32)


if __name__ == "__main__":
    x = np.random.randn(B, T, P).astype(np.float32)
    Drr = np.random.rand(N_POLES).astype(np.float32)
    Dtheta = np.random.rand(N_POLES).astype(np.float32)
    o = kernel(x, Drr, Dtheta)
    print(o.shape, o.dtype)


# revision 16
# speedup vs baseline: 1.3073x; 1.0447x over previous
"""FISTA sparse-coding encoder kernel for Trainium2 (8 NeuronCores).

Problem: x [2,10,20480] f32, Drr/Dtheta [40] f32.
  D = normalized dictionary [10, 161]
  A = I - D^T D / L,  DtY = D^T Y / L,  lam = gamma / L
  40 FISTA iterations: xn = softshrink(A @ y + DtY); y = xn + m (xn - x_old)
  output sparsecode [2, 161, 20480].

Design ("u-form", v4: fp16 matmuls + PSUM-packed tails):
  - Data-parallel over columns: Y reshaped to [10, 40960]; 5120 columns/core.
  - Momentum identity: A y_i + DtY = (1+m) u_i - m u_{i-1} with
    u_i = A x_i + DtY, so each iteration needs one matmul pass over x_i and
    one fused elementwise op  xn = shrink(s0*u_i - m*u_{i-1}, lam).
  - Output rows split 118 (head) + 43 (tail).  Contraction split 128 + 43:
    block 1 = [x rows 0..117 ; Y glue rows] (DtY via the glue), block 2 =
    x rows 118..160 packed at partition bases 0/64 of xT.
  - Tail packing: the two 512-column halves of a 1024-column group write
    their 43 tail rows into ONE [107, 512] PSUM tile at partition offsets 0
    and 64 (PE quad-tile col positions; half A's tail out-block is 64 wide
    with 21 zero weight columns so the pad partitions are initialized).
    The tail DVE/ACT ops then process 1024 columns in 512 free-cycles,
    cutting elementwise busy/iter from 2N to 1.5N.  PSUM partition offsets
    require 16-bit matmul operands (walrus rejects f32r quad-tiling), so
    x-state and weights are fp16: u stays f32 in PSUM, the momentum
    combines f32 u's, and only x is rounded; measured end-to-end error
    ~9e-3 vs the f32 reference (tolerance 2e-2).
  - Per iteration per group:
      PE:  8 matmuls -> u head [118,1024] + u tail packed [107,512] (f32)
      DVE: fused shrink+momentum from PSUM + raw-u SBUF -> fp16 x state
      ACT: raw copies u -> uoH/uoT (single buffer; DVE reads the old value
           first, write-after-read ordered by the tile framework)
"""

import numpy as np

# ---------------------------------------------------------------- constants
B, T, N_POLES, P = 2, 10, 40, 20480
MAX_ITER = 40
GAMMA = 0.01
K = 4 * N_POLES + 1          # 161
NCORES = 8
NCOLS = B * P // NCORES      # 5120 columns per core
GRP = 1024                   # column group (head psum = 2 banks)
HB = 512                     # half-block (one PSUM bank, matmul free dim)
NGRP = NCOLS // GRP          # 5
KH = 118                     # head out rows (x rows 0..117)
KT = K - KH                  # 43 tail out rows (x rows 118..160)
PCOLS = NCOLS // 2           # 2560 packed tail columns
TB = 64                      # packed-tail half B base partition
XT_P = TB + KT               # 107 partitions of packed tail state

_cache = {}


# ------------------------------------------------------------- custom DVE op
def _register_shrinkmom3():
    """out = relu(w - C2) + min(w + C2, 0)  with  w = in0*s0 + in1*s1.

    Softshrink of an affine combination of two raw tensors:
    s0 = (1+m_prev), s1 = -m_prev, imm2 = lam.  8 ALU stages.
    """
    import concourse.dve_ops as dve_ops
    from concourse.dve_spec import (
        Spec, Src0, Src1, C0, C1, C2, Zero, relu, minn, lower,
    )
    from concourse.dve_spec import _has_src1 as has_src1
    from concourse.dve_uop import DveOpSpec

    name = "ANT_SHRINKMOM3_FISTA"
    if any(op.name == name for op in dve_ops.OPS):
        return next(op for op in dve_ops.OPS if op.name == name)

    w = Src0 * C0 + Src1 * C1
    spec = Spec(
        body=relu(w - C2) + minn(w + C2, Zero),
        reference=lambda in0, in1, s0=1.0, s1=0.0, imm2=0.0: (
            lambda ww: (np.maximum(ww - imm2, 0.0)
                        + np.minimum(ww + imm2, 0.0)).astype(np.float32)
        )(in0 * s0 + in1 * s1),
    )
    op = dve_ops.DveOp(name, spec, subdim=False, uops_sha={})
    dve_ops.OPS.append(op)
    dve_ops.CUSTOM_DVE_SPECS[name] = spec
    dve_ops._SUB_OPCODE_FOR_NAME[name] = (
        dve_ops._CUSTOM_DVE_ROW_BASE + len(dve_ops.OPS) - 1
    )
    for ver in ("v3", "v4"):
        compiled = DveOpSpec(
            name=name,
            opcode=dve_ops.get_dve_sub_opcode(name),
            uops=lower(spec, ver=ver),
            rd1_en=has_src1(spec),
        )
        op.uops_sha[ver] = compiled.sha(ver)
    return op


# ------------------------------------------------------------ host constants
def _host_constants(Drr, Dtheta):
    r = Drr.astype(np.float64)
    th = Dtheta.astype(np.float64)
    i = np.arange(T, dtype=np.float64)[:, None]
    pr = r[None, :] ** i
    sgn = np.where(np.arange(T)[:, None] % 2 == 0, 1.0, -1.0)
    c = np.cos(i * th[None, :])
    s = np.sin(i * th[None, :])
    ones = np.ones((T, 1))
    dic = np.concatenate([ones, pr * c, sgn * pr * c, pr * s, sgn * pr * s], axis=1)
    G = np.linalg.norm(dic, axis=0)
    G = np.where(G == 0, np.sqrt(float(T)), G)
    D = (dic / G).astype(np.float32)            # [T, K]

    D64 = D.astype(np.float64)
    DtD = D64.T @ D64
    L = float(np.linalg.norm(DtD))              # Frobenius
    A = np.eye(K) - DtD / L                     # [K, K] (symmetric)
    lam = float(GAMMA / L)

    # contraction block 1 rows: [A rows 0..117 ; D/L glue rows]  [128, K]
    w1 = np.concatenate([A[0:KH, :], D64 / L], axis=0).astype(np.float32)
    # contraction block 2 rows: A rows 118..160  [43, K]
    w2 = A[KH:K, :].astype(np.float32)

    # momentum coefficients m_i = (t_i - 1)/t_{i+1}, t_0 = 1
    ms = []
    t = 1.0
    for _ in range(MAX_ITER):
        t_new = (1.0 + np.sqrt(1.0 + 4.0 * t * t)) / 2.0
        ms.append((t - 1.0) / t_new)
        t = t_new
    return w1, w2, lam, ms


# ------------------------------------------------------------- bass program
def _build_program():
    import concourse.mybir as mybir
    import concourse.tile as tile
    from concourse import bacc

    fused_op = _register_shrinkmom3()

    f32 = mybir.dt.float32
    f16 = mybir.dt.float16

    nc = bacc.Bacc("TRN2", target_bir_lowering=False, debug=False,
                   num_devices=NCORES)

    ycols = nc.dram_tensor("ycols", [T, NCOLS], f32, kind="ExternalInput")
    d_l1a = nc.dram_tensor("l1a", [128, KH], f32, kind="ExternalInput")
    # tail-out weights for half A carry 21 zero columns so the matmul also
    # zero-fills the pad partitions 43..63 of the packed PSUM tile (free —
    # matmul cost depends only on the moving free size)
    d_l1b = nc.dram_tensor("l1b", [128, TB], f32, kind="ExternalInput")
    d_l2a = nc.dram_tensor("l2a", [KT, KH], f32, kind="ExternalInput")
    d_l2b = nc.dram_tensor("l2b", [KT, TB], f32, kind="ExternalInput")
    out = nc.dram_tensor("out", [K, NCOLS], f32, kind="ExternalOutput")

    lam, ms = _cache["consts_meta"]

    with tile.TileContext(nc) as tc:
        with (
            tc.tile_pool(name="state", bufs=1) as st,
            tc.tile_pool(name="wts", bufs=1) as wts,
            tc.tile_pool(name="psH", bufs=3, space="PSUM") as psH,
            tc.tile_pool(name="psT", bufs=2, space="PSUM") as psT,
        ):
            # ---- persistent state -------------------------------------
            # xH rows 0..117 = x head; rows 118..127 = Y glue (written once)
            xH = [st.tile([128, NCOLS], f16, tag=f"xH{b}", name=f"xH{b}")
                  for b in range(2)]
            # packed tail: half A at partitions 0..42, half B at 64..106
            xT = [st.tile([XT_P, PCOLS], f16, tag=f"xT{b}", name=f"xT{b}")
                  for b in range(2)]
            # raw u_{i-1} staging (single buffer: fused op reads before the
            # ACT copy overwrites)
            uoH = st.tile([KH, NCOLS], f32, tag="uoH", name="uoH")
            uoT = st.tile([XT_P, PCOLS], f32, tag="uoT", name="uoT")
            # f32 copies of the final iterate for the output DMA
            foH = st.tile([KH, NCOLS], f32, tag="foH", name="foH")
            foT = st.tile([XT_P, PCOLS], f32, tag="foT", name="foT")

            # fp32 staging for DMA'd weights -> fp16 copies
            ws1 = wts.tile([128, KH + TB], f32, tag="ws1", name="ws1")
            ws2 = wts.tile([KT, KH + TB], f32, tag="ws2", name="ws2")
            l1a = wts.tile([128, KH], f16, tag="l1a", name="l1a")
            l1b = wts.tile([128, TB], f16, tag="l1b", name="l1b")
            # block-2 weights duplicated at partition bases 0 and 64
            l2a = wts.tile([XT_P, KH], f16, tag="l2a", name="l2a")
            l2b = wts.tile([XT_P, TB], f16, tag="l2b", name="l2b")

            nc.sync.dma_start(ws1[:, 0:KH], d_l1a[:])
            nc.sync.dma_start(ws1[:, KH:KH + TB], d_l1b[:])
            nc.sync.dma_start(ws2[:, 0:KH], d_l2a[:])
            nc.sync.dma_start(ws2[:, KH:KH + TB], d_l2b[:])
            nc.scalar.copy(l1a[:], ws1[:, 0:KH])
            nc.scalar.copy(l1b[:], ws1[:, KH:KH + TB])
            nc.scalar.copy(l2a[0:KT, :], ws2[:, 0:KH])
            nc.scalar.copy(l2b[0:KT, :], ws2[:, KH:KH + TB])
            nc.scalar.copy(l2a[TB:TB + KT, :], ws2[:, 0:KH])
            nc.scalar.copy(l2b[TB:TB + KT, :], ws2[:, KH:KH + TB])

            # ---- init ------------------------------------------------
            # x_0 = 0 (so iteration 0 computes u_0 = DtY from the glue
            # alone); uo = 0.  Zeros + glue-Y staged in f32 and
            # engine-copied (converting) into the fp16 state.  Everything
            # is chunked by column group so iteration 0's first matmuls
            # start as soon as group 0's state is ready instead of after
            # the whole init.
            with tc.tile_pool(name="init", bufs=1) as ip:
                zst = ip.tile([128, NCOLS], f32, tag="zst", name="zst")
                for g in range(NGRP):
                    gs = slice(g * GRP, (g + 1) * GRP)
                    pc = slice(g * HB, (g + 1) * HB)
                    nc.gpsimd.memset(zst[0:KH, gs], 0.0)
                    nc.sync.dma_start(zst[KH:128, gs], ycols[:, gs])
                    nc.scalar.copy(xH[0][:, gs], zst[:, gs])
                    nc.vector.tensor_copy(xT[0][:, pc],
                                          zst[0:XT_P, g * HB:(g + 1) * HB])
                    nc.gpsimd.memset(uoH[:, gs], 0.0)
                    nc.gpsimd.memset(uoT[:, pc], 0.0)
                # xH[1] only needs its glue rows before iteration 1 reads
                # them (rows 0..117 are written by iteration 0's DVE)
                nc.vector.tensor_copy(xH[1][96:128, :], zst[96:128, :])

            def mm(ps, lhsT, rhs, start, stop):
                nc.tensor.matmul(ps, lhsT, rhs, start=start, stop=stop,
                                 skip_group_check=True)

            for it in range(MAX_ITER):
                m_prev = ms[it - 1] if it > 0 else 0.0
                s0 = float(1.0 + m_prev)
                s1 = float(-m_prev)
                cur, nxt = it % 2, (it + 1) % 2
                xc_h, xc_t = xH[cur], xT[cur]
                xn_h, xn_t = xH[nxt], xT[nxt]
                last = it == MAX_ITER - 1

                for g in range(NGRP):
                    gs = slice(g * GRP, (g + 1) * GRP)
                    pc = slice(g * HB, (g + 1) * HB)
                    csA = slice(g * GRP, g * GRP + HB)
                    csB = slice(g * GRP + HB, (g + 1) * GRP)

                    wh = psH.tile([KH, GRP], mybir.dt.float32, tag="wh",
                                  name="wh")
                    wt = psT.tile([XT_P, HB], mybir.dt.float32, tag="wt",
                                  name="wt")

                    # half A: contraction block 2 at partition base 0; tail
                    # out-block 64 wide (43 real + 21 zero) -> pads written
                    mm(wh[:, 0:HB], l1a[:], xc_h[:, csA], True, False)
                    mm(wh[:, 0:HB], l2a[0:KT, :], xc_t[0:KT, pc], False, True)
                    mm(wt[0:TB, :], l1b[:], xc_h[:, csA], True, False)
                    mm(wt[0:TB, :], l2b[0:KT, :], xc_t[0:KT, pc], False, True)
                    # half B: tail lands at PSUM partition base 64
                    mm(wh[:, HB:GRP], l1a[:], xc_h[:, csB], True, False)
                    mm(wh[:, HB:GRP], l2a[TB:TB + KT, :],
                       xc_t[TB:TB + KT, pc], False, True)
                    mm(wt[TB:TB + KT, :], l1b[:, 0:KT], xc_h[:, csB],
                       True, False)
                    mm(wt[TB:TB + KT, :], l2b[TB:TB + KT, 0:KT],
                       xc_t[TB:TB + KT, pc], False, True)

                    # fused momentum + soft-threshold (reads uo = u_{i-1}
                    # BEFORE the ACT copies below overwrite it).  The final
                    # iterate goes to f32 tiles for the output DMA instead
                    # of the fp16 state (nothing consumes x_40 on-chip).
                    oh = foH[:, gs] if last else xn_h[0:KH, gs]
                    ot = foT[:, pc] if last else xn_t[:, pc]
                    nc.vector._custom_dve(fused_op, out=oh,
                                          in0=wh[:], in1=uoH[:, gs],
                                          s0=s0, s1=s1, imm2=float(lam))
                    nc.vector._custom_dve(fused_op, out=ot,
                                          in0=wt[:], in1=uoT[:, pc],
                                          s0=s0, s1=s1, imm2=float(lam))

                    if last:
                        nc.sync.dma_start(out[0:KH, gs], foH[:, gs])
                        nc.sync.dma_start(out[KH:K, csA], foT[0:KT, pc])
                        nc.sync.dma_start(out[KH:K, csB],
                                          foT[TB:TB + KT, pc])
                    else:
                        nc.scalar.copy(uoH[:, gs], wh[:])
                        nc.scalar.copy(uoT[:, pc], wt[:])
    nc.finalize()
    return nc


def _get_program(lam, ms):
    key = (round(lam, 12), tuple(round(m, 9) for m in ms))
    if _cache.get("key") != key:
        _cache["consts_meta"] = (lam, ms)
        _cache["nc"] = _build_program()
        _cache["key"] = key
    return _cache["nc"]


# ------------------------------------------------------------------- kernel
def kernel(x, Drr, Dtheta):
    from concourse.bass_utils import run_bass_kernel_spmd

    w1, w2, lam, ms = _host_constants(Drr, Dtheta)
    nc = _get_program(lam, ms)

    l1a = np.ascontiguousarray(w1[:, 0:KH])
    l1b = np.zeros((128, TB), np.float32)
    l1b[:, 0:KT] = w1[:, KH:K]
    l2a = np.ascontiguousarray(w2[:, 0:KH])
    l2b = np.zeros((KT, TB), np.float32)
    l2b[:, 0:KT] = w2[:, KH:K]

    xc = np.ascontiguousarray(
        np.transpose(x.astype(np.float32), (1, 0, 2)).reshape(T, B * P))

    in_maps = []
    for c in range(NCORES):
        in_maps.append({
            "ycols": np.ascontiguousarray(xc[:, c * NCOLS:(c + 1) * NCOLS]),
            "l1a": l1a, "l1b": l1b, "l2a": l2a, "l2b": l2b,
        })

    res = run_bass_kernel_spmd(nc, in_maps, core_ids=list(range(NCORES)))
    _cache["last_res"] = res
    full = np.concatenate([r["out"] for r in res.results], axis=1)  # [K, B*P]
    return np.ascontiguousarray(
        full.reshape(K, B, P).transpose(1, 0, 2)).astype(np.float32)


if __name__ == "__main__":
    x = np.random.randn(B, T, P).astype(np.float32)
    Drr = np.random.rand(N_POLES).astype(np.float32)
    Dtheta = np.random.rand(N_POLES).astype(np.float32)
    o = kernel(x, Drr, Dtheta)
    print(o.shape, o.dtype)


# revision 19
# speedup vs baseline: 1.3501x; 1.0328x over previous
"""FISTA sparse-coding encoder kernel for Trainium2 (8 NeuronCores).

Problem: x [2,10,20480] f32, Drr/Dtheta [40] f32.
  D = normalized dictionary [10, 161]
  A = I - D^T D / L,  DtY = D^T Y / L,  lam = gamma / L
  40 FISTA iterations: xn = softshrink(A @ y + DtY); y = xn + m (xn - x_old)
  output sparsecode [2, 161, 20480].

Design ("u-form", v4: fp16 matmuls + PSUM-packed tails):
  - Data-parallel over columns: Y reshaped to [10, 40960]; 5120 columns/core.
  - Momentum identity: A y_i + DtY = (1+m) u_i - m u_{i-1} with
    u_i = A x_i + DtY, so each iteration needs one matmul pass over x_i and
    one fused elementwise op  xn = shrink(s0*u_i - m*u_{i-1}, lam).
  - Output rows split 118 (head) + 43 (tail).  Contraction split 128 + 43:
    block 1 = [x rows 0..117 ; Y glue rows] (DtY via the glue), block 2 =
    x rows 118..160 packed at partition bases 0/64 of xT.
  - Tail packing: the two 512-column halves of a 1024-column group write
    their 43 tail rows into ONE [107, 512] PSUM tile at partition offsets 0
    and 64 (PE quad-tile col positions; half A's tail out-block is 64 wide
    with 21 zero weight columns so the pad partitions are initialized).
    The tail DVE/ACT ops then process 1024 columns in 512 free-cycles,
    cutting elementwise busy/iter from 2N to 1.5N.  PSUM partition offsets
    require 16-bit matmul operands (walrus rejects f32r quad-tiling), so
    x-state and weights are fp16: u stays f32 in PSUM, the momentum
    combines f32 u's, and only x is rounded; measured end-to-end error
    ~9e-3 vs the f32 reference (tolerance 2e-2).
  - Per iteration per group:
      PE:  8 matmuls -> u head [118,1024] + u tail packed [107,512] (f32)
      DVE: fused shrink+momentum from PSUM + raw-u SBUF -> fp16 x state
      ACT: raw copies u -> uoH/uoT (single buffer; DVE reads the old value
           first, write-after-read ordered by the tile framework)
"""

import numpy as np

# ---------------------------------------------------------------- constants
B, T, N_POLES, P = 2, 10, 40, 20480
MAX_ITER = 40
GAMMA = 0.01
K = 4 * N_POLES + 1          # 161
NCORES = 8
NCOLS = B * P // NCORES      # 5120 columns per core
GRP = 1024                   # column group (head psum = 2 banks)
HB = 512                     # half-block (one PSUM bank, matmul free dim)
NGRP = NCOLS // GRP          # 5
KH = 118                     # head out rows (x rows 0..117)
KT = K - KH                  # 43 tail out rows (x rows 118..160)
PCOLS = NCOLS // 2           # 2560 packed tail columns
TB = 64                      # packed-tail half B base partition
XT_P = TB + KT               # 107 partitions of packed tail state

_cache = {}


# ------------------------------------------------------------- custom DVE op
def _register_shrinkmom3():
    """out = relu(w - C2) + min(w + C2, 0)  with  w = in0*s0 + in1*s1.

    Softshrink of an affine combination of two raw tensors:
    s0 = (1+m_prev), s1 = -m_prev, imm2 = lam.  8 ALU stages.
    """
    import concourse.dve_ops as dve_ops
    from concourse.dve_spec import (
        Spec, Src0, Src1, C0, C1, C2, Zero, relu, minn, lower,
    )
    from concourse.dve_spec import _has_src1 as has_src1
    from concourse.dve_uop import DveOpSpec

    name = "ANT_SHRINKMOM3_FISTA"
    if any(op.name == name for op in dve_ops.OPS):
        return next(op for op in dve_ops.OPS if op.name == name)

    w = Src0 * C0 + Src1 * C1
    spec = Spec(
        body=relu(w - C2) + minn(w + C2, Zero),
        reference=lambda in0, in1, s0=1.0, s1=0.0, imm2=0.0: (
            lambda ww: (np.maximum(ww - imm2, 0.0)
                        + np.minimum(ww + imm2, 0.0)).astype(np.float32)
        )(in0 * s0 + in1 * s1),
    )
    op = dve_ops.DveOp(name, spec, subdim=False, uops_sha={})
    dve_ops.OPS.append(op)
    dve_ops.CUSTOM_DVE_SPECS[name] = spec
    dve_ops._SUB_OPCODE_FOR_NAME[name] = (
        dve_ops._CUSTOM_DVE_ROW_BASE + len(dve_ops.OPS) - 1
    )
    for ver in ("v3", "v4"):
        compiled = DveOpSpec(
            name=name,
            opcode=dve_ops.get_dve_sub_opcode(name),
            uops=lower(spec, ver=ver),
            rd1_en=has_src1(spec),
        )
        op.uops_sha[ver] = compiled.sha(ver)
    return op


# ------------------------------------------------------------ host constants
def _host_constants(Drr, Dtheta):
    r = Drr.astype(np.float64)
    th = Dtheta.astype(np.float64)
    i = np.arange(T, dtype=np.float64)[:, None]
    pr = r[None, :] ** i
    sgn = np.where(np.arange(T)[:, None] % 2 == 0, 1.0, -1.0)
    c = np.cos(i * th[None, :])
    s = np.sin(i * th[None, :])
    ones = np.ones((T, 1))
    dic = np.concatenate([ones, pr * c, sgn * pr * c, pr * s, sgn * pr * s], axis=1)
    G = np.linalg.norm(dic, axis=0)
    G = np.where(G == 0, np.sqrt(float(T)), G)
    D = (dic / G).astype(np.float32)            # [T, K]

    D64 = D.astype(np.float64)
    DtD = D64.T @ D64
    L = float(np.linalg.norm(DtD))              # Frobenius
    A = np.eye(K) - DtD / L                     # [K, K] (symmetric)
    lam = float(GAMMA / L)

    # contraction block 1 rows: [A rows 0..117 ; D/L glue rows]  [128, K]
    w1 = np.concatenate([A[0:KH, :], D64 / L], axis=0).astype(np.float32)
    # contraction block 2 rows: A rows 118..160  [43, K]
    w2 = A[KH:K, :].astype(np.float32)

    # momentum coefficients m_i = (t_i - 1)/t_{i+1}, t_0 = 1
    ms = []
    t = 1.0
    for _ in range(MAX_ITER):
        t_new = (1.0 + np.sqrt(1.0 + 4.0 * t * t)) / 2.0
        ms.append((t - 1.0) / t_new)
        t = t_new
    return w1, w2, lam, ms


# ------------------------------------------------------------- bass program
def _build_program():
    import concourse.mybir as mybir
    import concourse.tile as tile
    from concourse import bacc

    fused_op = _register_shrinkmom3()

    f32 = mybir.dt.float32
    f16 = mybir.dt.float16

    nc = bacc.Bacc("TRN2", target_bir_lowering=False, debug=False,
                   num_devices=NCORES)

    ycols = nc.dram_tensor("ycols", [T, NCOLS], f32, kind="ExternalInput")
    d_l1a = nc.dram_tensor("l1a", [128, KH], f32, kind="ExternalInput")
    # tail-out weights for half A carry 21 zero columns so the matmul also
    # zero-fills the pad partitions 43..63 of the packed PSUM tile (free —
    # matmul cost depends only on the moving free size)
    d_l1b = nc.dram_tensor("l1b", [128, TB], f32, kind="ExternalInput")
    d_l2a = nc.dram_tensor("l2a", [KT, KH], f32, kind="ExternalInput")
    d_l2b = nc.dram_tensor("l2b", [KT, TB], f32, kind="ExternalInput")
    out = nc.dram_tensor("out", [K, NCOLS], f32, kind="ExternalOutput")

    lam, ms = _cache["consts_meta"]

    with tile.TileContext(nc) as tc:
        with (
            tc.tile_pool(name="state", bufs=1) as st,
            tc.tile_pool(name="wts", bufs=1) as wts,
            tc.tile_pool(name="psH", bufs=3, space="PSUM") as psH,
            tc.tile_pool(name="psT", bufs=2, space="PSUM") as psT,
        ):
            # ---- persistent state -------------------------------------
            # xH rows 0..117 = x head; rows 118..127 = Y glue (written once)
            xH = [st.tile([128, NCOLS], f16, tag=f"xH{b}", name=f"xH{b}")
                  for b in range(2)]
            # packed tail: half A at partitions 0..42, half B at 64..106
            xT = [st.tile([XT_P, PCOLS], f16, tag=f"xT{b}", name=f"xT{b}")
                  for b in range(2)]
            # raw u evacuations, double-buffered: esX[it%2] holds u_it; the
            # fused op reads the current one as in0 (all-SBUF is cheaper
            # for the DVE than PSUM) and the previous one as in1
            esH = [st.tile([KH, NCOLS], f32, tag=f"esH{b}", name=f"esH{b}")
                   for b in range(2)]
            esT = [st.tile([XT_P, PCOLS], f32, tag=f"esT{b}", name=f"esT{b}")
                   for b in range(2)]
            # f32 copies of the final iterate for the output DMA
            foH = st.tile([KH, NCOLS], f32, tag="foH", name="foH")
            foT = st.tile([XT_P, PCOLS], f32, tag="foT", name="foT")

            # fp32 staging for DMA'd weights -> fp16 copies
            ws1 = wts.tile([128, KH + TB], f32, tag="ws1", name="ws1")
            ws2 = wts.tile([KT, KH + TB], f32, tag="ws2", name="ws2")
            l1a = wts.tile([128, KH], f16, tag="l1a", name="l1a")
            l1b = wts.tile([128, TB], f16, tag="l1b", name="l1b")
            # block-2 weights duplicated at partition bases 0 and 64
            l2a = wts.tile([XT_P, KH], f16, tag="l2a", name="l2a")
            l2b = wts.tile([XT_P, TB], f16, tag="l2b", name="l2b")

            nc.sync.dma_start(ws1[:, 0:KH], d_l1a[:])
            nc.sync.dma_start(ws1[:, KH:KH + TB], d_l1b[:])
            nc.sync.dma_start(ws2[:, 0:KH], d_l2a[:])
            nc.sync.dma_start(ws2[:, KH:KH + TB], d_l2b[:])
            nc.scalar.copy(l1a[:], ws1[:, 0:KH])
            nc.scalar.copy(l1b[:], ws1[:, KH:KH + TB])
            nc.scalar.copy(l2a[0:KT, :], ws2[:, 0:KH])
            nc.scalar.copy(l2b[0:KT, :], ws2[:, KH:KH + TB])
            nc.scalar.copy(l2a[TB:TB + KT, :], ws2[:, 0:KH])
            nc.scalar.copy(l2b[TB:TB + KT, :], ws2[:, KH:KH + TB])

            # ---- init ------------------------------------------------
            # x_0 = 0 (so iteration 0 computes u_0 = DtY from the glue
            # alone); uo = 0.  Zeros + glue-Y staged in f32 and
            # engine-copied (converting) into the fp16 state.  Everything
            # is chunked by column group so iteration 0's first matmuls
            # start as soon as group 0's state is ready instead of after
            # the whole init.
            with tc.tile_pool(name="init", bufs=1) as ip:
                zst = ip.tile([128, NCOLS], f32, tag="zst", name="zst")
                for g in range(NGRP):
                    gs = slice(g * GRP, (g + 1) * GRP)
                    pc = slice(g * HB, (g + 1) * HB)
                    nc.gpsimd.memset(zst[0:KH, gs], 0.0)
                    nc.sync.dma_start(zst[KH:128, gs], ycols[:, gs])
                    nc.scalar.copy(xH[0][:, gs], zst[:, gs])
                    nc.gpsimd.tensor_copy(xT[0][:, pc],
                                          zst[0:XT_P, g * HB:(g + 1) * HB])
                    nc.gpsimd.memset(esH[1][:, gs], 0.0)
                    nc.gpsimd.memset(esT[1][:, pc], 0.0)
                # xH[1] only needs its glue rows before iteration 1 reads
                # them (rows 0..117 are written by iteration 0's DVE)
                nc.gpsimd.tensor_copy(xH[1][96:128, :], zst[96:128, :])

            def mm(ps, lhsT, rhs, start, stop):
                nc.tensor.matmul(ps, lhsT, rhs, start=start, stop=stop,
                                 skip_group_check=True)

            for it in range(MAX_ITER):
                m_prev = ms[it - 1] if it > 0 else 0.0
                s0 = float(1.0 + m_prev)
                s1 = float(-m_prev)
                cur, nxt = it % 2, (it + 1) % 2
                xc_h, xc_t = xH[cur], xT[cur]
                xn_h, xn_t = xH[nxt], xT[nxt]
                last = it == MAX_ITER - 1

                for g in range(NGRP):
                    gs = slice(g * GRP, (g + 1) * GRP)
                    pc = slice(g * HB, (g + 1) * HB)
                    csA = slice(g * GRP, g * GRP + HB)
                    csB = slice(g * GRP + HB, (g + 1) * GRP)

                    wh = psH.tile([KH, GRP], mybir.dt.float32, tag="wh",
                                  name="wh")
                    wt = psT.tile([XT_P, HB], mybir.dt.float32, tag="wt",
                                  name="wt")

                    # half A: contraction block 2 at partition base 0; tail
                    # out-block 64 wide (43 real + 21 zero) -> pads written
                    mm(wh[:, 0:HB], l1a[:], xc_h[:, csA], True, False)
                    mm(wh[:, 0:HB], l2a[0:KT, :], xc_t[0:KT, pc], False, True)
                    mm(wt[0:TB, :], l1b[:], xc_h[:, csA], True, False)
                    mm(wt[0:TB, :], l2b[0:KT, :], xc_t[0:KT, pc], False, True)
                    # half B: tail lands at PSUM partition base 64
                    mm(wh[:, HB:GRP], l1a[:], xc_h[:, csB], True, False)
                    mm(wh[:, HB:GRP], l2a[TB:TB + KT, :],
                       xc_t[TB:TB + KT, pc], False, True)
                    mm(wt[TB:TB + KT, :], l1b[:, 0:KT], xc_h[:, csB],
                       True, False)
                    mm(wt[TB:TB + KT, :], l2b[TB:TB + KT, 0:KT],
                       xc_t[TB:TB + KT, pc], False, True)

                    # evacuate raw u to SBUF (frees the PSUM tile and gives
                    # the fused op an all-SBUF read path)
                    nc.scalar.copy(esH[cur][:, gs], wh[:])
                    nc.scalar.copy(esT[cur][:, pc], wt[:])

                    # fused momentum + soft-threshold.  The final iterate
                    # goes to f32 tiles for the output DMA instead of the
                    # fp16 state (nothing consumes x_40 on-chip).
                    oh = foH[:, gs] if last else xn_h[0:KH, gs]
                    ot = foT[:, pc] if last else xn_t[:, pc]
                    nc.vector._custom_dve(fused_op, out=oh,
                                          in0=esH[cur][:, gs],
                                          in1=esH[nxt][:, gs],
                                          s0=s0, s1=s1, imm2=float(lam))
                    nc.vector._custom_dve(fused_op, out=ot,
                                          in0=esT[cur][:, pc],
                                          in1=esT[nxt][:, pc],
                                          s0=s0, s1=s1, imm2=float(lam))

                    if last:
                        nc.sync.dma_start(out[0:KH, gs], foH[:, gs])
                        nc.sync.dma_start(out[KH:K, csA], foT[0:KT, pc])
                        nc.sync.dma_start(out[KH:K, csB],
                                          foT[TB:TB + KT, pc])
    nc.finalize()
    return nc


def _get_program(lam, ms):
    key = (round(lam, 12), tuple(round(m, 9) for m in ms))
    if _cache.get("key") != key:
        _cache["consts_meta"] = (lam, ms)
        _cache["nc"] = _build_program()
        _cache["key"] = key
    return _cache["nc"]


# ------------------------------------------------------------------- kernel
def kernel(x, Drr, Dtheta):
    from concourse.bass_utils import run_bass_kernel_spmd

    w1, w2, lam, ms = _host_constants(Drr, Dtheta)
    nc = _get_program(lam, ms)

    l1a = np.ascontiguousarray(w1[:, 0:KH])
    l1b = np.zeros((128, TB), np.float32)
    l1b[:, 0:KT] = w1[:, KH:K]
    l2a = np.ascontiguousarray(w2[:, 0:KH])
    l2b = np.zeros((KT, TB), np.float32)
    l2b[:, 0:KT] = w2[:, KH:K]

    xc = np.ascontiguousarray(
        np.transpose(x.astype(np.float32), (1, 0, 2)).reshape(T, B * P))

    in_maps = []
    for c in range(NCORES):
        in_maps.append({
            "ycols": np.ascontiguousarray(xc[:, c * NCOLS:(c + 1) * NCOLS]),
            "l1a": l1a, "l1b": l1b, "l2a": l2a, "l2b": l2b,
        })

    res = run_bass_kernel_spmd(nc, in_maps, core_ids=list(range(NCORES)))
    _cache["last_res"] = res
    full = np.concatenate([r["out"] for r in res.results], axis=1)  # [K, B*P]
    return np.ascontiguousarray(
        full.reshape(K, B, P).transpose(1, 0, 2)).astype(np.float32)


if __name__ == "__main__":
    x = np.random.randn(B, T, P).astype(np.float32)
    Drr = np.random.rand(N_POLES).astype(np.float32)
    Dtheta = np.random.rand(N_POLES).astype(np.float32)
    o = kernel(x, Drr, Dtheta)
    print(o.shape, o.dtype)
